# revision 5
# baseline (speedup 1.0000x reference)
"""Trainium2 Bass kernel for the MultiHeadAttention transformer block.

Sharding: 8 cores, core c handles batch b=c//2 and query-row half
(c%2)*1024 .. +1024, all 8 heads.  Each core is fully independent
(no collectives): it computes Q/K/V projections (Q only for its row
half, K/V for the full 2048 keys), masked softmax attention,
residual+LN, fc, residual+LN for its 1024 rows and writes its
[1024, 512] slice of the output.

Host-side prep: transpose+cast q,k,v and the weights to bf16 so all
on-chip matmul operands arrive with the contraction dim on partitions.
Scores are computed transposed (S^T[lk, lq]) so the key-mask is a
per-partition bias folded into the Exp activation, and the softmax
row-sums come for free out of the A@V matmul via an appended
ones-column on V.
"""

import sys

if "/opt/trn_rl_repo" not in sys.path:
    sys.path.insert(0, "/opt/trn_rl_repo")

import numpy as np

import concourse.bacc as bacc
import concourse.bass as bass
import concourse.tile as tile
from concourse import mybir
from concourse.bass_utils import run_bass_kernel_spmd

H, D, DK, DV = 8, 512, 64, 64
B, L = 4, 2048
P = 128
LQ = L // 2          # query rows per core
NCORES = 8
EPS = 1e-5
NEG = -1e9 / 8.0     # masked score after the /temperature divide
F32 = mybir.dt.float32
BF16 = mybir.dt.bfloat16
AF = mybir.ActivationFunctionType

_CACHE = {}


def _emit(nc, tc):
    DT = D // P   # 4 d-tiles
    CH = D // P   # 4 channel chunks (H*DK = 512)
    LKT = L // P  # 16 key tiles
    RQ = LQ // P  # 8 query row chunks

    qT = nc.dram_tensor("qT", [D, LQ], BF16, kind="ExternalInput")
    kT = nc.dram_tensor("kT", [D, L], BF16, kind="ExternalInput")
    vT = nc.dram_tensor("vT", [D, L], BF16, kind="ExternalInput")
    qres = nc.dram_tensor("qres", [LQ, D], F32, kind="ExternalInput")
    WqT = nc.dram_tensor("WqT", [D, D], BF16, kind="ExternalInput")
    WkT = nc.dram_tensor("WkT", [D, D], BF16, kind="ExternalInput")
    WvT = nc.dram_tensor("WvT", [D, D], BF16, kind="ExternalInput")
    fcwT = nc.dram_tensor("fcwT", [D, D], BF16, kind="ExternalInput")
    mb = nc.dram_tensor("mb", [P, H * LKT], F32, kind="ExternalInput")
    vecs = nc.dram_tensor("vecs", [5, D], F32, kind="ExternalInput")
    out = nc.dram_tensor("out", [LQ, D], F32, kind="ExternalOutput")

    def bcast_row(row_ap, dst):
        src = bass.AP(tensor=row_ap.tensor, offset=row_ap.offset,
                      ap=[[0, P], [1, D]])
        nc.gpsimd.dma_start(out=dst, in_=src)

    with (
        tc.tile_pool(name="consts", bufs=1) as consts,
        tc.tile_pool(name="psum", bufs=2, space="PSUM") as psum,
        tc.tile_pool(name="projout", bufs=1) as projout,
        tc.tile_pool(name="small", bufs=6) as small,
    ):
        # ---- constants resident for the whole kernel ----
        mb_s = consts.tile([P, H * LKT], F32)
        nc.sync.dma_start(out=mb_s, in_=mb[:, :])
        fcwT_s = consts.tile([P, DT, D], BF16)
        nc.sync.dma_start(out=fcwT_s, in_=fcwT[:, :].rearrange("(t p) n -> p t n", p=P))
        g0b = consts.tile([P, D], F32)
        b0b = consts.tile([P, D], F32)
        g1b = consts.tile([P, D], F32)
        b1b = consts.tile([P, D], F32)
        fcbb = consts.tile([P, D], F32)
        for i, dst in enumerate((g0b, b0b, g1b, b1b, fcbb)):
            bcast_row(vecs[i], dst)
        eps_t = consts.tile([P, 1], F32)
        nc.vector.memset(eps_t, EPS)

        # ---- persistent projection outputs ----
        QT_s = projout.tile([P, CH, LQ], BF16)     # Q^T  [ch, lq]
        KT_s = projout.tile([P, CH, L], BF16)      # K^T  [ch, lk]
        V_s = projout.tile([P, LKT, H, DV + 1], BF16)  # V + ones col
        attn_s = projout.tile([P, RQ, D], F32)     # attention concat out

        # ================= phase A: projections =================
        with tc.tile_pool(name="inp", bufs=1) as inp, \
             tc.tile_pool(name="wts", bufs=1) as wts:
            qT_s = inp.tile([P, DT, LQ], BF16)
            nc.sync.dma_start(out=qT_s, in_=qT[:, :].rearrange("(t p) n -> p t n", p=P))
            kT_s = inp.tile([P, DT, L], BF16)
            nc.sync.dma_start(out=kT_s, in_=kT[:, :].rearrange("(t p) n -> p t n", p=P))
            vT_s = inp.tile([P, DT, L], BF16)
            nc.sync.dma_start(out=vT_s, in_=vT[:, :].rearrange("(t p) n -> p t n", p=P))
            WqT_s = wts.tile([P, DT, D], BF16)
            nc.sync.dma_start(out=WqT_s, in_=WqT[:, :].rearrange("(t p) n -> p t n", p=P))
            WkT_s = wts.tile([P, DT, D], BF16)
            nc.sync.dma_start(out=WkT_s, in_=WkT[:, :].rearrange("(t p) n -> p t n", p=P))
            WvT_s = wts.tile([P, DT, D], BF16)
            nc.sync.dma_start(out=WvT_s, in_=WvT[:, :].rearrange("(t p) n -> p t n", p=P))

            # Q^T[ch, lq] = sum_d WqT[d, ch] qT[d, lq]
            for m in range(CH):
                for jb in range(LQ // 512):
                    ps = psum.tile([P, 512], F32, tag="proj")
                    for dt in range(DT):
                        nc.tensor.matmul(
                            ps[:, :],
                            WqT_s[:, dt, m * P:(m + 1) * P],
                            qT_s[:, dt, jb * 512:(jb + 1) * 512],
                            start=(dt == 0), stop=(dt == DT - 1))
                    nc.vector.tensor_copy(QT_s[:, m, jb * 512:(jb + 1) * 512], ps[:, :])
            # K^T[ch, lk]
            for m in range(CH):
                for jb in range(L // 512):
                    ps = psum.tile([P, 512], F32, tag="proj")
                    for dt in range(DT):
                        nc.tensor.matmul(
                            ps[:, :],
                            WkT_s[:, dt, m * P:(m + 1) * P],
                            kT_s[:, dt, jb * 512:(jb + 1) * 512],
                            start=(dt == 0), stop=(dt == DT - 1))
                    nc.vector.tensor_copy(KT_s[:, m, jb * 512:(jb + 1) * 512], ps[:, :])
            # V[lk, ch] (natural orientation, interleaved with ones col)
            nc.vector.memset(V_s[:, :, :, DV:DV + 1], 1.0)
            for lk in range(LKT):
                ps = psum.tile([P, 512], F32, tag="proj")
                for dt in range(DT):
                    nc.tensor.matmul(
                        ps[:, :],
                        vT_s[:, dt, lk * P:(lk + 1) * P],
                        WvT_s[:, dt, :],
                        start=(dt == 0), stop=(dt == DT - 1))
                nc.vector.tensor_copy(
                    V_s[:, lk, :, 0:DV],
                    ps[:, :].rearrange("p (h e) -> p h e", h=H))

        # ================= phase B: attention =================
        with tc.tile_pool(name="pT", bufs=24) as pTp:
            for h in range(H):
                mt = h // 2
                po = (h % 2) * DK
                pts = []
                for m in range(LKT):
                    ps = psum.tile([P, LQ], F32, tag="qk")
                    for jb in range(LQ // 512):
                        nc.tensor.matmul(
                            ps[:, jb * 512:(jb + 1) * 512],
                            KT_s[po:po + DK, mt, m * P:(m + 1) * P],
                            QT_s[po:po + DK, mt, jb * 512:(jb + 1) * 512],
                            start=True, stop=True)
                    pt = pTp.tile([P, LQ], BF16, tag="pT")
                    nc.scalar.activation(
                        out=pt[:, :], in_=ps[:, :], func=AF.Exp,
                        bias=mb_s[:, h * LKT + m:h * LKT + m + 1],
                        scale=1.0 / 8.0)
                    pts.append(pt)
                for s in range(RQ):
                    ps = psum.tile([P, DV + 1], F32, tag="av")
                    for t in range(LKT):
                        nc.tensor.matmul(
                            ps[:, :],
                            pts[t][:, s * P:(s + 1) * P],
                            V_s[:, t, h, :],
                            start=(t == 0), stop=(t == LKT - 1))
                    rc = small.tile([P, 1], F32, tag="recip")
                    nc.vector.reciprocal(rc, ps[:, DV:DV + 1])
                    nc.vector.tensor_scalar_mul(
                        out=attn_s[:, s, h * DV:(h + 1) * DV],
                        in0=ps[:, 0:DV], scalar1=rc)

        # ================= phase C: residual + LN0 =================
        with tc.tile_pool(name="ln", bufs=1) as ln, \
             tc.tile_pool(name="work", bufs=4) as work:
            LN0f = ln.tile([P, RQ, D], F32)
            LN0bf = ln.tile([P, RQ, D], BF16)
            LN0T = ln.tile([P, DT, LQ], BF16)

            def layer_norm_rows(x0, gb, bb, out_ap):
                st = small.tile([P, 6], F32, tag="st")
                nc.vector.bn_stats(out=st, in_=x0)
                mv = small.tile([P, 2], F32, tag="mv")
                nc.vector.bn_aggr(out=mv, in_=st)
                sd = small.tile([P, 1], F32, tag="sd")
                nc.scalar.activation(out=sd, in_=mv[:, 1:2], func=AF.Sqrt,
                                     bias=eps_t[:, :])
                rs = small.tile([P, 1], F32, tag="rs")
                nc.vector.reciprocal(rs, sd)
                xn = work.tile([P, D], F32, tag="xn")
                nc.vector.tensor_scalar(
                    out=xn, in0=x0, scalar1=mv[:, 0:1], scalar2=rs,
                    op0=mybir.AluOpType.subtract, op1=mybir.AluOpType.mult)
                nc.vector.tensor_mul(xn, xn, gb)
                nc.vector.tensor_add(out_ap, xn, bb)

            for r in range(RQ):
                qr = work.tile([P, D], F32, tag="qres")
                nc.sync.dma_start(out=qr, in_=qres[r * P:(r + 1) * P, :])
                x0 = work.tile([P, D], F32, tag="x0")
                nc.vector.tensor_add(x0, attn_s[:, r, :], qr)
                layer_norm_rows(x0, g0b, b0b, LN0f[:, r, :])
                nc.vector.tensor_copy(LN0bf[:, r, :], LN0f[:, r, :])
                for dt in range(DT):
                    nc.sync.dma_start_transpose(
                        out=LN0T[:, dt, r * P:(r + 1) * P],
                        in_=LN0bf[:, r, dt * P:(dt + 1) * P])

            # ============= phase D: fc + residual + LN1 =============
            for r in range(RQ):
                ps = psum.tile([P, 512], F32, tag="proj")
                for dt in range(DT):
                    nc.tensor.matmul(
                        ps[:, :],
                        LN0T[:, dt, r * P:(r + 1) * P],
                        fcwT_s[:, dt, :],
                        start=(dt == 0), stop=(dt == DT - 1))
                y0 = work.tile([P, D], F32, tag="y0")
                nc.vector.tensor_add(y0, ps[:, :], fcbb)
                nc.vector.tensor_add(y0, y0, LN0f[:, r, :])
                ot = work.tile([P, D], F32, tag="ot")
                layer_norm_rows(y0, g1b, b1b, ot)
                nc.sync.dma_start(out=out[r * P:(r + 1) * P, :], in_=ot)


def _build():
    if "nc" in _CACHE:
        return _CACHE["nc"]
    nc = bacc.Bacc(None, target_bir_lowering=False, debug=False)
    with tile.TileContext(nc) as tc:
        _emit(nc, tc)
    nc.compile()
    _CACHE["nc"] = nc
    return nc


def _prep_in_maps(q, k, v, mask, Wq, Wk, Wv, fc_w, fc_b, g0, b0, g1, b1):
    q = np.asarray(q, np.float32)
    k = np.asarray(k, np.float32)
    v = np.asarray(v, np.float32)
    mask = np.asarray(mask)
    bf = mybir.dt.np(BF16)

    WqTh = np.ascontiguousarray(np.asarray(Wq, np.float32).T).astype(bf)
    WkTh = np.ascontiguousarray(np.asarray(Wk, np.float32).T).astype(bf)
    WvTh = np.ascontiguousarray(np.asarray(Wv, np.float32).T).astype(bf)
    fcwTh = np.ascontiguousarray(np.asarray(fc_w, np.float32).T).astype(bf)
    vecs = np.stack([np.asarray(x, np.float32) for x in (g0, b0, g1, b1, fc_b)])

    in_maps = []
    for c in range(NCORES):
        b = c // 2
        r0 = (c % 2) * LQ
        qTb = np.ascontiguousarray(q[b].T[:, r0:r0 + LQ]).astype(bf)
        kTb = np.ascontiguousarray(k[b].T).astype(bf)
        vTb = np.ascontiguousarray(v[b].T).astype(bf)
        qrb = np.ascontiguousarray(q[b][r0:r0 + LQ])
        mbh = np.zeros((P, H, L // P), np.float32)
        for h in range(H):
            mh = mask[h * B + b].reshape(L // P, P).T  # [p, tile]
            mbh[:, h, :] = np.where(mh == 0, np.float32(NEG), np.float32(0.0))
        in_maps.append({
            "qT": qTb, "kT": kTb, "vT": vTb, "qres": qrb,
            "WqT": WqTh, "WkT": WkTh, "WvT": WvTh, "fcwT": fcwTh,
            "mb": np.ascontiguousarray(mbh.reshape(P, H * (L // P))),
            "vecs": vecs,
        })
    return in_maps


def kernel(q, k, v, mask, Wq, Wk, Wv, fc_w, fc_b, g0, b0, g1, b1):
    in_maps = _prep_in_maps(q, k, v, mask, Wq, Wk, Wv, fc_w, fc_b,
                            g0, b0, g1, b1)
    nc = _build()
    res = run_bass_kernel_spmd(nc, in_maps, core_ids=list(range(NCORES)))
    outf = np.empty((B, L, D), np.float32)
    for c in range(NCORES):
        b = c // 2
        r0 = (c % 2) * LQ
        outf[b, r0:r0 + LQ, :] = res.results[c]["out"]
    return outf
